# revision 49
# baseline (speedup 1.0000x reference)
"""Ewald potential Bass kernels for TRN2 (8-core SPMD), v2.

K1 shards k-space (480 cols/core of padded 3840) over all 8192 atoms ->
akp=|k_pot| and v_pot (re/im). Host gathers. K2 shards atoms (1024/core):
aw GEMM -> softmax -> inverse transform, out.T per core.

v2 vs baseline:
- All heavy GEMMs (k_pot/v_pot in K1, aw in K2) run at full PE rate via
  float32r operands (HW-probed: ~1.3e-4 rel err, 4x faster than fp32).
- K2 pass-2 sm transposes moved from DMA (240 serializing DMA_TRANSPOSEs)
  to PE transposes into PSUM.
- Magic-number round offloaded to GPSIMD; sin+cos fused into a single
  ACT call on a packed [p, 2*K] tile.

out[n,d] = sum_k sm[n,k] * (cos(ph_i)*vpr[k,d] + sin(ph_i)*vpi[k,d]) / Z[n]
"""
import sys
sys.path.insert(0, '/opt/trn_rl_repo')
import numpy as np
import ml_dtypes
import concourse.bass as bass
import concourse.tile as tile
import concourse.mybir as mybir
from concourse import bacc
from concourse.bass_utils import run_bass_kernel_spmd
from concourse.masks import make_identity
from contextlib import ExitStack

F = mybir.ActivationFunctionType
DT = mybir.dt
ALU = mybir.AluOpType
AX = mybir.AxisListType

P = 128
N = 8192
D = 128
KPAD = 3840          # 3796 padded to 30*128
AWK = 3840           # aw/sm width (= KPAD); pass-1 computes it in 2x1920 halves
AWH = 1920           # aw half width (4 PSUM banks)
KSH = KPAD // 8      # 480 k-cols per core in K1
NSH = N // 8         # 1024 atoms per core in K2
NCH = N // P         # 64 atom chunks in K1
KCH = KPAD // P      # 30 k chunks in K2
NC2 = NSH // P       # 8 atom chunks in K2
MAGIC = 12582912.0   # 1.5 * 2^23
TWOPI = float(2 * np.pi)

bf16 = ml_dtypes.bfloat16

# 'f32r': pot matmuls use float32r (sincos produced as f32r by ACT)
# 'f16' : pot matmuls use float16 (kv/vv cast on host, sincos f16 by ACT)
POT_MODE = 'f32r'
POT_DT = DT.float32r if POT_MODE == 'f32r' else DT.float16
POT_NP = np.float32 if POT_MODE == 'f32r' else np.float16


def host_prep(q_vector, k_vector, v_vector, positions, cell, k_fwd, k_inv):
    """Per-axis centered-frac tables + 0/1 selection matrices.

    phase'[n,j] = sum_axis frac(k_axis[j] * rfrac[n,axis]) in [-1.5, 1.5];
    on device one range-wrap recovers the centered fractional phase. The
    table rides the phase matmul as [th; tl] bf16 split (126 rows) against
    the duplicated selection matrix [S; S]."""
    L = float(np.asarray(cell).reshape(3, 3)[0, 0])
    rf = (np.asarray(positions, dtype=np.float32) / np.float32(L))
    rf = rf.astype(np.float64)                              # [N,3]
    K = k_fwd.shape[0]
    kx = np.arange(13)
    kyz = np.arange(-12, 13)
    t = np.concatenate([rf[:, [0]] * kx, rf[:, [1]] * kyz, rf[:, [2]] * kyz],
                       axis=1)                              # [N,63]
    t = t - np.round(t)                                     # [-0.5, 0.5]
    tT = t.T
    th = tT.astype(bf16)
    tl = (tT - th.astype(np.float64)).astype(bf16)
    tstk = np.ascontiguousarray(np.concatenate([th, tl], axis=0))  # [126,N]

    def smat(kmat):  # [K,3] effective exponents -> [126, KPAD] bf16 0/1
        S = np.zeros((63, KPAD), dtype=np.float32)
        j = np.arange(K)
        S[kmat[:, 0], j] = 1
        S[13 + kmat[:, 1] + 12, j] = 1
        S[38 + kmat[:, 2] + 12, j] = 1
        return np.ascontiguousarray(
            np.concatenate([S, S], axis=0).astype(bf16))

    Sfwd = smat(np.asarray(k_fwd).astype(int))
    Sinv = smat(np.asarray(k_inv).astype(int))
    qT_abs = np.abs(np.asarray(q_vector, dtype=np.float32)).T.copy()  # [128,N]
    kv = np.ascontiguousarray(np.asarray(k_vector, dtype=POT_NP))
    vv = np.ascontiguousarray(np.asarray(v_vector, dtype=POT_NP))
    return tstk, Sfwd, Sinv, qT_abs, kv, vv, K


# ---------------------------------------------------------------- kernel 1
def build_k1():
    nc = bacc.Bacc("TRN2", target_bir_lowering=False, debug=False)
    rsp_d = nc.dram_tensor("tstk", [126, N], DT.bfloat16, kind="ExternalInput").ap()
    ktab_d = nc.dram_tensor("sfwd", [126, KSH], DT.bfloat16, kind="ExternalInput").ap()
    kv_d = nc.dram_tensor("kv", [N, D], POT_DT, kind="ExternalInput").ap()
    vv_d = nc.dram_tensor("vv", [N, D], POT_DT, kind="ExternalInput").ap()
    akp_d = nc.dram_tensor("akp", [D, KSH], DT.float32, kind="ExternalOutput").ap()
    vpr_d = nc.dram_tensor("vpr", [D, KSH], DT.bfloat16, kind="ExternalOutput").ap()
    vpi_d = nc.dram_tensor("vpi", [D, KSH], DT.bfloat16, kind="ExternalOutput").ap()

    with ExitStack() as ctx:
        tc = ctx.enter_context(tile.TileContext(nc))
        cpool = ctx.enter_context(tc.tile_pool(name="const", bufs=1))
        wpool = ctx.enter_context(tc.tile_pool(name="work", bufs=3))
        pspool = ctx.enter_context(tc.tile_pool(name="ph", bufs=2, space="PSUM"))
        acc_ps = ctx.enter_context(tc.tile_pool(name="acc", bufs=1, space="PSUM"))

        rsp = cpool.tile([126, N], DT.bfloat16)
        ktab = cpool.tile([126, KSH], DT.bfloat16)
        kv = cpool.tile([P, NCH * D], POT_DT)    # [128 n-part, chunk-major d]
        vv = cpool.tile([P, NCH * D], POT_DT)
        nc.sync.dma_start(rsp[:], rsp_d)
        nc.sync.dma_start(ktab[:], ktab_d)
        # big kv/vv loads: split in two and issue from the (idle) ACT and
        # GPSIMD queues so descriptor generation runs in parallel with the
        # sync queue instead of serializing ~20us of kernel start.
        kv_r = kv[:].rearrange("p (c d) -> p c d", d=D)
        vv_r = vv[:].rearrange("p (c d) -> p c d", d=D)
        kvd_r = kv_d.rearrange("(c p) d -> p c d", p=P)
        vvd_r = vv_d.rearrange("(c p) d -> p c d", p=P)
        for i in range(2):
            cs = slice(i * (NCH // 2), (i + 1) * (NCH // 2))
            nc.scalar.dma_start(kv_r[:, cs], kvd_r[:, cs])
            nc.gpsimd.dma_start(vv_r[:, cs], vvd_r[:, cs])

        kre = acc_ps.tile([P, KSH], DT.float32)
        kim = acc_ps.tile([P, KSH], DT.float32)
        vre = acc_ps.tile([P, KSH], DT.float32)
        vim = acc_ps.tile([P, KSH], DT.float32)

        # paired atom chunks: one trig chain per 2 chunks. ph holds 2 chunk
        # phases at 512-col stride (PSUM bank align). The ph matmuls are
        # emitted one block AHEAD of the trig/pot stage so the PE queue never
        # gates the DVE chain (software pipelining).
        NB = NCH // 2

        def emit_ph(b):
            ph = pspool.tile([P, 1024], DT.float32, tag="ph")
            for half in range(2):
                c = 2 * b + half
                nc.tensor.matmul(ph[:, half * 512:half * 512 + KSH],
                                 rsp[:, c * P:(c + 1) * P], ktab[:],
                                 start=True, stop=True)
            return ph

        prev = emit_ph(0)
        for b in range(NB):
            nxt = emit_ph(b + 1) if b + 1 < NB else None
            ph = prev
            phv = ph[:].rearrange("p (a b) -> p a b", a=2)[:, :, :KSH]
            sc = wpool.tile([P, 4 * KSH], DT.float32, tag="sc")
            scv = sc[:, :2 * KSH].rearrange("p (a b) -> p a b", a=2)
            nc.vector.add_range_wrap(scv, phv, 0.0, 0.5, 1.0)
            nc.vector.add_range_wrap(sc[:, 2 * KSH:], sc[:, :2 * KSH],
                                     0.25, 0.5, 1.0)
            sincos = wpool.tile([P, 4 * KSH], POT_DT, tag="sincos")
            nc.scalar.activation(sincos[:], sc[:], F.Sin, scale=TWOPI)
            for half in range(2):
                c = 2 * b + half
                sin = sincos[:, half * KSH:(half + 1) * KSH]
                cos = sincos[:, (2 + half) * KSH:(3 + half) * KSH]
                st = dict(start=(c == 0), stop=(c == NCH - 1))
                kvc = kv[:, c * D:(c + 1) * D]
                vvc = vv[:, c * D:(c + 1) * D]
                nc.tensor.matmul(kre[:], kvc, cos, **st)
                nc.tensor.matmul(kim[:], kvc, sin, **st)
                nc.tensor.matmul(vre[:], vvc, cos, **st)
                nc.tensor.matmul(vim[:], vvc, sin, **st)
            prev = nxt

        # akp = sqrt(kre^2 + kim^2)
        sq1 = wpool.tile([P, KSH], DT.float32, tag="sq1")
        sq2 = wpool.tile([P, KSH], DT.float32, tag="sq2")
        nc.scalar.activation(sq1[:], kre[:], F.Square)
        nc.scalar.activation(sq2[:], kim[:], F.Square)
        ssum = wpool.tile([P, KSH], DT.float32, tag="ssum")
        nc.vector.tensor_add(ssum[:], sq1[:], sq2[:])
        akp = wpool.tile([P, KSH], DT.float32, tag="akp")
        nc.scalar.activation(akp[:], ssum[:], F.Sqrt)
        nc.sync.dma_start(akp_d, akp[:])
        vrb = wpool.tile([P, KSH], DT.bfloat16, tag="vrb")
        vib = wpool.tile([P, KSH], DT.bfloat16, tag="vib")
        nc.vector.tensor_copy(vrb[:], vre[:])
        nc.vector.tensor_copy(vib[:], vim[:])
        nc.sync.dma_start(vpr_d, vrb[:])
        nc.sync.dma_start(vpi_d, vib[:])

    nc.compile()
    return nc


# ---------------------------------------------------------------- kernel 2
def build_k2():
    nc = bacc.Bacc("TRN2", target_bir_lowering=False, debug=False)
    rsp_d = nc.dram_tensor("tloc", [126, NSH], DT.bfloat16,
                           kind="ExternalInput").ap()
    ktab_d = nc.dram_tensor("sinv", [126, KPAD], DT.bfloat16,
                            kind="ExternalInput").ap()
    qT_d = nc.dram_tensor("qT", [D, NSH], DT.float32r, kind="ExternalInput").ap()
    akp_d = nc.dram_tensor("akp", [D, AWK], DT.float32r, kind="ExternalInput").ap()
    vprT_d = nc.dram_tensor("vprT", [KPAD, D], DT.bfloat16,
                            kind="ExternalInput").ap()
    vpiT_d = nc.dram_tensor("vpiT", [KPAD, D], DT.bfloat16,
                            kind="ExternalInput").ap()
    outT0_d = nc.dram_tensor("outT0", [D, NSH], DT.float32,
                             kind="ExternalOutput").ap()
    outT1_d = nc.dram_tensor("outT1", [D, NSH], DT.float32,
                             kind="ExternalOutput").ap()
    zs_d = nc.dram_tensor("zs", [P, 2 * NC2], DT.float32, kind="ExternalOutput").ap()
    mx_d = nc.dram_tensor("mxs", [P, 2 * NC2], DT.float32, kind="ExternalOutput").ap()

    with ExitStack() as ctx:
        tc = ctx.enter_context(tile.TileContext(nc))
        cpool = ctx.enter_context(tc.tile_pool(name="const", bufs=1))
        smpool = ctx.enter_context(tc.tile_pool(name="sm", bufs=1))
        wpool = ctx.enter_context(tc.tile_pool(name="work", bufs=2))
        zpool = ctx.enter_context(tc.tile_pool(name="z", bufs=1))

        rsp = cpool.tile([126, NSH], DT.bfloat16)
        ktab = cpool.tile([126, KPAD], DT.bfloat16)
        qT = cpool.tile([D, NSH], DT.float32r)
        akp = cpool.tile([D, AWK], DT.float32r)
        vprT = cpool.tile([P, KCH * D], DT.bfloat16)  # [128 k-part, chunk-major d]
        vpiT = cpool.tile([P, KCH * D], DT.bfloat16)
        ident = cpool.tile([P, P], DT.bfloat16)
        make_identity(nc, ident[:])
        # issue input DMAs from separate engine queues (parallel DGE)
        nc.sync.dma_start(rsp[:], rsp_d)
        nc.sync.dma_start(ktab[:], ktab_d)
        nc.gpsimd.dma_start(qT[:], qT_d)
        nc.scalar.dma_start(akp[:], akp_d)
        nc.gpsimd.dma_start(vprT[:].rearrange("p (c d) -> p c d", d=D),
                            vprT_d.rearrange("(c p) d -> p c d", p=P))
        nc.scalar.dma_start(vpiT[:].rearrange("p (c d) -> p c d", d=D),
                            vpiT_d.rearrange("(c p) d -> p c d", p=P))

        sm = smpool.tile([P, NC2 * AWK], DT.bfloat16)  # [128 n-part, chunk-major k]
        zacc = zpool.tile([P, 2 * NC2], DT.float32)    # [z_half0 | z_half1]
        mxout = zpool.tile([P, 2 * NC2], DT.float32)   # [-m_half0 | -m_half1]

        # ---- pass 1: aw -> unnormalized softmax per atom chunk, computed in
        # two INDEPENDENT 1920-col halves (separate max shift per half, merged
        # on the host). Halves ping-pong 4-bank PSUM tiles and have no cross
        # dependency, so the pipeline period is one reduce+exp, not a chunk.
        with tc.tile_pool(name="awps", bufs=2, space="PSUM") as awps:
            PIECES = (0, 512, 1024, 1536, AWH)
            for c in range(NC2):
                for h in range(2):
                    aw = awps.tile([P, AWH], DT.float32, tag="aw")
                    for j in range(4):
                        nc.tensor.matmul(
                            aw[:, PIECES[j]:PIECES[j + 1]],
                            qT[:, c * P:(c + 1) * P],
                            akp[:, h * AWH + PIECES[j]:h * AWH + PIECES[j + 1]],
                            start=True, stop=True)
                    negmx = mxout[:, h * NC2 + c:h * NC2 + c + 1]
                    nc.vector.reduce_max(negmx, aw[:], axis=AX.X, negate=True)
                    nc.scalar.activation(
                        sm[:, c * AWK + h * AWH:c * AWK + (h + 1) * AWH],
                        aw[:], F.Exp, bias=negmx,
                        accum_out=zacc[:, h * NC2 + c:h * NC2 + c + 1])

        # z and -m are final once pass 1 ends; DMA them out under pass 2
        nc.sync.dma_start(zs_d, zacc[:])
        nc.sync.dma_start(mx_d, mxout[:])

        # ---- pass 2: eik_i (transposed layout) + inverse transform. Two
        # sequential out.T accumulators matching the pass-1 halves (kc<15 ->
        # outT0 with shift m0, kc>=15 -> outT1 with m1); host merges.
        with (tc.tile_pool(name="phps", bufs=2, space="PSUM") as phps,
              tc.tile_pool(name="smps", bufs=2, space="PSUM") as smps,
              tc.tile_pool(name="ops", bufs=1, space="PSUM") as ops):
            HK = KCH // 2  # 15 kc per half

            def emit_front(kc):
                """ph matmuls + sm transposes for chunk kc (PE-only work,
                emitted one chunk ahead so the PE queue never blocks the
                DVE chain's inputs behind dependent out-matmuls)."""
                ph = phps.tile([P, NSH], DT.float32, tag="ph")
                for h in range(2):
                    nc.tensor.matmul(ph[:, h * 512:(h + 1) * 512],
                                     ktab[:, kc * P:(kc + 1) * P],
                                     rsp[:, h * 512:(h + 1) * 512],
                                     start=True, stop=True)
                smT = smps.tile([P, NSH], DT.bfloat16, tag="smT")
                for c in range(NC2):
                    nc.tensor.transpose(
                        smT[:, c * P:(c + 1) * P],
                        sm[:, c * AWK + kc * P: c * AWK + (kc + 1) * P],
                        ident[:])
                return ph, smT

            prev = emit_front(0)
            for half in range(2):
                outT = ops.tile([P, NSH], DT.float32, tag="outT")
                for kk in range(HK):
                    kc = half * HK + kk
                    nxt = emit_front(kc + 1) if kc + 1 < KCH else None
                    ph, smT = prev
                    # ACT copies ph to SBUF so the magic round runs in DVE 2x
                    # dual-port mode (PSUM operands disable it)
                    phs = wpool.tile([P, NSH], DT.float32, tag="phs")
                    nc.scalar.activation(phs[:], ph[:], F.Copy)
                    tr = wpool.tile([P, NSH], DT.float32, tag="tr")
                    nc.vector.tensor_scalar(tr[:], phs[:], MAGIC, MAGIC,
                                            ALU.add, ALU.subtract)
                    sc = wpool.tile([P, 2 * NSH], DT.float32, tag="sc")
                    nc.vector.scalar_tensor_tensor(sc[:, :NSH], tr[:], 0.0,
                                                   phs[:], ALU.add,
                                                   ALU.subtract)
                    nc.vector.add_range_wrap(sc[:, NSH:], sc[:, :NSH],
                                             -0.25, 0.5, 1.0)
                    sincos = wpool.tile([P, 2 * NSH], DT.bfloat16,
                                        tag="sincos")
                    nc.scalar.activation(sincos[:], sc[:], F.Sin, scale=-TWOPI)
                    smC = wpool.tile([P, NSH], DT.bfloat16, tag="smC")
                    smS = wpool.tile([P, NSH], DT.bfloat16, tag="smS")
                    nc.vector.tensor_mul(smC[:], smT[:], sincos[:, NSH:])
                    nc.vector.tensor_mul(smS[:], smT[:], sincos[:, :NSH])
                    # out.T += vprT_c.T @ smC + vpiT_c.T @ smS
                    for h in range(2):
                        hs = slice(h * 512, (h + 1) * 512)
                        nc.tensor.matmul(outT[:, hs],
                                         vprT[:, kc * D:(kc + 1) * D],
                                         smC[:, hs], start=(kk == 0),
                                         stop=False)
                        nc.tensor.matmul(outT[:, hs],
                                         vpiT[:, kc * D:(kc + 1) * D],
                                         smS[:, hs], start=False,
                                         stop=(kk == HK - 1))
                    prev = nxt
                res = wpool.tile([P, NSH], DT.float32, tag="res")
                nc.vector.tensor_copy(res[:], outT[:])
                nc.sync.dma_start(outT0_d if half == 0 else outT1_d, res[:])

    nc.compile()
    return nc


# ---------------------------------------------------------------- profiling
def enable_ntff_profiling():
    """Provide the antenv.axon_hooks module run_bass_kernel_spmd needs for
    trace=True under axon, backed by trn_boot's ctypes NTFF hook."""
    import types
    if "antenv.axon_hooks" in sys.modules:
        return True
    sys.path.insert(0, "/root/.axon_site")
    try:
        from trn_agent_boot.trn_boot import _ntff_profile_via_ctypes
        hook = _ntff_profile_via_ctypes("/opt/axon/libaxon_pjrt.so")
    except Exception as e:
        print(f"ntff hook unavailable: {e}")
        return False
    if hook is None:
        print("ntff hook: .so lacks axon_start_nrt_profile")
        return False
    mod = types.ModuleType("antenv.axon_hooks")
    mod._hook = hook
    mod.get_axon_ntff_profile_hook = lambda: mod._hook
    mod.set_axon_ntff_profile_hook = lambda h: setattr(mod, "_hook", h)
    sys.modules["antenv.axon_hooks"] = mod
    # upload_artifacts copies the NEFF dir to a remote bucket -- hangs in
    # this container; keep artifacts local instead.
    import concourse.bass_utils as bu
    bu.upload_artifacts = lambda tmpdir: tmpdir
    return True


# ---------------------------------------------------------------- runner
_NC1 = None
_NC2 = None


def run_ewald(q_vector, k_vector, v_vector, positions, cell, batch, k_fwd,
              k_inv, trace=False):
    global _NC1, _NC2
    if trace:
        trace = enable_ntff_profiling()
    tstk, Sfwd, Sinv, qT_abs, kv, vv, K = host_prep(
        q_vector, k_vector, v_vector, positions, cell, k_fwd, k_inv)

    if _NC1 is None:
        _NC1 = build_k1()
    in1 = [{"tstk": tstk,
            "sfwd": np.ascontiguousarray(Sfwd[:, c * KSH:(c + 1) * KSH]),
            "kv": kv, "vv": vv} for c in range(8)]
    r1 = run_bass_kernel_spmd(_NC1, in1, list(range(8)), trace=trace)

    akp = np.concatenate([r1.results[c]["akp"] for c in range(8)], axis=1)
    vpr = np.concatenate([r1.results[c]["vpr"] for c in range(8)], axis=1)
    vpi = np.concatenate([r1.results[c]["vpi"] for c in range(8)], axis=1)
    akp[:, K:] = 0.0
    akp_pad = np.ascontiguousarray(akp)  # AWK == KPAD
    vprT = np.ascontiguousarray(vpr.T)  # [KPAD, 128] bf16
    vpiT = np.ascontiguousarray(vpi.T)
    vprT[K:, :] = 0
    vpiT[K:, :] = 0

    if _NC2 is None:
        _NC2 = build_k2()
    in2 = [{"tloc": np.ascontiguousarray(tstk[:, c * NSH:(c + 1) * NSH]),
            "sinv": Sinv,
            "qT": np.ascontiguousarray(qT_abs[:, c * NSH:(c + 1) * NSH]),
            "akp": akp_pad, "vprT": vprT, "vpiT": vpiT} for c in range(8)]
    r2 = run_bass_kernel_spmd(_NC2, in2, list(range(8)), trace=trace)

    outs = []
    for c in range(8):
        o0 = r2.results[c]["outT0"]              # [128 d, 1024 n], shift m0
        o1 = r2.results[c]["outT1"]              # [128 d, 1024 n], shift m1
        zh = r2.results[c]["zs"]                 # [128, 16] = [z_h0 | z_h1]
        mh = r2.results[c]["mxs"]                # [128, 16] = [-m_h0 | -m_h1]
        m0 = -mh[:, :NC2].T.reshape(-1).astype(np.float64)
        m1 = -mh[:, NC2:].T.reshape(-1).astype(np.float64)
        m = np.maximum(m0, m1)
        s0 = np.exp(m0 - m)
        s1 = np.exp(m1 - m)
        z = (zh[:, :NC2].T.reshape(-1) * s0 + zh[:, NC2:].T.reshape(-1) * s1)
        num = o0.T * s0[:, None] + o1.T * s1[:, None]
        outs.append((num / z[:, None]).astype(np.float32))
    out = np.concatenate(outs, axis=0)
    return out, (r1, r2)


# ---------------------------------------------------------------- entry point
def kernel(q_vector, k_vector, v_vector, positions, cell, batch, k_fwd, k_inv):
    """Full-input entry: shards across 8 NeuronCores internally."""
    out, _ = run_ewald(np.asarray(q_vector), np.asarray(k_vector),
                       np.asarray(v_vector), np.asarray(positions),
                       np.asarray(cell), np.asarray(batch),
                       np.asarray(k_fwd), np.asarray(k_inv))
    return out


# revision 50
# speedup vs baseline: 1.0231x; 1.0231x over previous
"""Ewald potential Bass kernels for TRN2 (8-core SPMD), v2.

K1 shards k-space (480 cols/core of padded 3840) over all 8192 atoms ->
akp=|k_pot| and v_pot (re/im). Host gathers. K2 shards atoms (1024/core):
aw GEMM -> softmax -> inverse transform, out.T per core.

v2 vs baseline:
- All heavy GEMMs (k_pot/v_pot in K1, aw in K2) run at full PE rate via
  float32r operands (HW-probed: ~1.3e-4 rel err, 4x faster than fp32).
- K2 pass-2 sm transposes moved from DMA (240 serializing DMA_TRANSPOSEs)
  to PE transposes into PSUM.
- Magic-number round offloaded to GPSIMD; sin+cos fused into a single
  ACT call on a packed [p, 2*K] tile.

out[n,d] = sum_k sm[n,k] * (cos(ph_i)*vpr[k,d] + sin(ph_i)*vpi[k,d]) / Z[n]
"""
import sys
sys.path.insert(0, '/opt/trn_rl_repo')
import numpy as np
import ml_dtypes
import concourse.bass as bass
import concourse.tile as tile
import concourse.mybir as mybir
from concourse import bacc
from concourse.bass_utils import run_bass_kernel_spmd
from concourse.masks import make_identity
from contextlib import ExitStack

F = mybir.ActivationFunctionType
DT = mybir.dt
ALU = mybir.AluOpType
AX = mybir.AxisListType

P = 128
N = 8192
D = 128
KPAD = 3840          # 3796 padded to 30*128
AWK = 3840           # aw/sm width (= KPAD); pass-1 computes it in 2x1920 halves
AWH = 1920           # aw half width (4 PSUM banks)
KSH = KPAD // 8      # 480 k-cols per core in K1
NSH = N // 8         # 1024 atoms per core in K2
NCH = N // P         # 64 atom chunks in K1
KCH = KPAD // P      # 30 k chunks in K2
NC2 = NSH // P       # 8 atom chunks in K2
MAGIC = 12582912.0   # 1.5 * 2^23
TWOPI = float(2 * np.pi)

bf16 = ml_dtypes.bfloat16

# 'f32r': pot matmuls use float32r (sincos produced as f32r by ACT)
# 'f16' : pot matmuls use float16 (kv/vv cast on host, sincos f16 by ACT)
POT_MODE = 'f32r'
POT_DT = DT.float32r if POT_MODE == 'f32r' else DT.float16
POT_NP = np.float32 if POT_MODE == 'f32r' else np.float16


def host_prep(q_vector, k_vector, v_vector, positions, cell, k_fwd, k_inv):
    """Per-axis centered-frac tables + 0/1 selection matrices.

    phase'[n,j] = sum_axis frac(k_axis[j] * rfrac[n,axis]) in [-1.5, 1.5];
    on device one range-wrap recovers the centered fractional phase. The
    table rides the phase matmul as [th; tl] bf16 split (126 rows) against
    the duplicated selection matrix [S; S]."""
    L = float(np.asarray(cell).reshape(3, 3)[0, 0])
    rf = (np.asarray(positions, dtype=np.float32) / np.float32(L))
    rf = rf.astype(np.float64)                              # [N,3]
    K = k_fwd.shape[0]
    kx = np.arange(13)
    kyz = np.arange(-12, 13)
    t = np.concatenate([rf[:, [0]] * kx, rf[:, [1]] * kyz, rf[:, [2]] * kyz],
                       axis=1)                              # [N,63]
    t = t - np.round(t)                                     # [-0.5, 0.5]
    tT = t.T
    th = tT.astype(bf16)
    tl = (tT - th.astype(np.float64)).astype(bf16)
    tstk = np.ascontiguousarray(np.concatenate([th, tl], axis=0))  # [126,N]

    def smat(kmat):  # [K,3] effective exponents -> [126, KPAD] bf16 0/1
        S = np.zeros((63, KPAD), dtype=np.float32)
        j = np.arange(K)
        S[kmat[:, 0], j] = 1
        S[13 + kmat[:, 1] + 12, j] = 1
        S[38 + kmat[:, 2] + 12, j] = 1
        return np.ascontiguousarray(
            np.concatenate([S, S], axis=0).astype(bf16))

    Sfwd = smat(np.asarray(k_fwd).astype(int))
    Sinv = smat(np.asarray(k_inv).astype(int))
    qT_abs = np.abs(np.asarray(q_vector, dtype=np.float32)).T.copy()  # [128,N]
    kv = np.ascontiguousarray(np.asarray(k_vector, dtype=POT_NP))
    vv = np.ascontiguousarray(np.asarray(v_vector, dtype=POT_NP))
    return tstk, Sfwd, Sinv, qT_abs, kv, vv, K


# ---------------------------------------------------------------- kernel 1
def build_k1():
    nc = bacc.Bacc("TRN2", target_bir_lowering=False, debug=False)
    rsp_d = nc.dram_tensor("tstk", [126, N], DT.bfloat16, kind="ExternalInput").ap()
    ktab_d = nc.dram_tensor("sfwd", [126, KSH], DT.bfloat16, kind="ExternalInput").ap()
    kv_d = nc.dram_tensor("kv", [N, D], POT_DT, kind="ExternalInput").ap()
    vv_d = nc.dram_tensor("vv", [N, D], POT_DT, kind="ExternalInput").ap()
    akp_d = nc.dram_tensor("akp", [D, KSH], DT.float32, kind="ExternalOutput").ap()
    vpr_d = nc.dram_tensor("vpr", [D, KSH], DT.bfloat16, kind="ExternalOutput").ap()
    vpi_d = nc.dram_tensor("vpi", [D, KSH], DT.bfloat16, kind="ExternalOutput").ap()

    with ExitStack() as ctx:
        tc = ctx.enter_context(tile.TileContext(nc))
        cpool = ctx.enter_context(tc.tile_pool(name="const", bufs=1))
        wpool = ctx.enter_context(tc.tile_pool(name="work", bufs=3))
        pspool = ctx.enter_context(tc.tile_pool(name="ph", bufs=2, space="PSUM"))
        acc_ps = ctx.enter_context(tc.tile_pool(name="acc", bufs=1, space="PSUM"))

        rsp = cpool.tile([126, N], DT.bfloat16)
        ktab = cpool.tile([126, KSH], DT.bfloat16)
        kv = cpool.tile([P, NCH * D], POT_DT)    # [128 n-part, chunk-major d]
        vv = cpool.tile([P, NCH * D], POT_DT)
        nc.sync.dma_start(rsp[:], rsp_d)
        nc.sync.dma_start(ktab[:], ktab_d)
        # split the big kv/vv loads so compute can start after the first piece
        kv_r = kv[:].rearrange("p (c d) -> p c d", d=D)
        vv_r = vv[:].rearrange("p (c d) -> p c d", d=D)
        kvd_r = kv_d.rearrange("(c p) d -> p c d", p=P)
        vvd_r = vv_d.rearrange("(c p) d -> p c d", p=P)
        for i in range(4):
            cs = slice(i * (NCH // 4), (i + 1) * (NCH // 4))
            nc.sync.dma_start(kv_r[:, cs], kvd_r[:, cs])
            nc.sync.dma_start(vv_r[:, cs], vvd_r[:, cs])

        kre = acc_ps.tile([P, KSH], DT.float32)
        kim = acc_ps.tile([P, KSH], DT.float32)
        vre = acc_ps.tile([P, KSH], DT.float32)
        vim = acc_ps.tile([P, KSH], DT.float32)

        # paired atom chunks: one trig chain per 2 chunks. ph holds 2 chunk
        # phases at 512-col stride (PSUM bank align). The ph matmuls are
        # emitted one block AHEAD of the trig/pot stage so the PE queue never
        # gates the DVE chain (software pipelining).
        NB = NCH // 2

        def emit_ph(b):
            ph = pspool.tile([P, 1024], DT.float32, tag="ph")
            for half in range(2):
                c = 2 * b + half
                nc.tensor.matmul(ph[:, half * 512:half * 512 + KSH],
                                 rsp[:, c * P:(c + 1) * P], ktab[:],
                                 start=True, stop=True)
            return ph

        prev = emit_ph(0)
        for b in range(NB):
            nxt = emit_ph(b + 1) if b + 1 < NB else None
            ph = prev
            phv = ph[:].rearrange("p (a b) -> p a b", a=2)[:, :, :KSH]
            sc = wpool.tile([P, 4 * KSH], DT.float32, tag="sc")
            scv = sc[:, :2 * KSH].rearrange("p (a b) -> p a b", a=2)
            nc.vector.add_range_wrap(scv, phv, 0.0, 0.5, 1.0)
            nc.vector.add_range_wrap(sc[:, 2 * KSH:], sc[:, :2 * KSH],
                                     0.25, 0.5, 1.0)
            sincos = wpool.tile([P, 4 * KSH], POT_DT, tag="sincos")
            nc.scalar.activation(sincos[:], sc[:], F.Sin, scale=TWOPI)
            for half in range(2):
                c = 2 * b + half
                sin = sincos[:, half * KSH:(half + 1) * KSH]
                cos = sincos[:, (2 + half) * KSH:(3 + half) * KSH]
                st = dict(start=(c == 0), stop=(c == NCH - 1))
                kvc = kv[:, c * D:(c + 1) * D]
                vvc = vv[:, c * D:(c + 1) * D]
                nc.tensor.matmul(kre[:], kvc, cos, **st)
                nc.tensor.matmul(kim[:], kvc, sin, **st)
                nc.tensor.matmul(vre[:], vvc, cos, **st)
                nc.tensor.matmul(vim[:], vvc, sin, **st)
            prev = nxt

        # akp = sqrt(kre^2 + kim^2)
        sq1 = wpool.tile([P, KSH], DT.float32, tag="sq1")
        sq2 = wpool.tile([P, KSH], DT.float32, tag="sq2")
        nc.scalar.activation(sq1[:], kre[:], F.Square)
        nc.scalar.activation(sq2[:], kim[:], F.Square)
        ssum = wpool.tile([P, KSH], DT.float32, tag="ssum")
        nc.vector.tensor_add(ssum[:], sq1[:], sq2[:])
        akp = wpool.tile([P, KSH], DT.float32, tag="akp")
        nc.scalar.activation(akp[:], ssum[:], F.Sqrt)
        nc.sync.dma_start(akp_d, akp[:])
        vrb = wpool.tile([P, KSH], DT.bfloat16, tag="vrb")
        vib = wpool.tile([P, KSH], DT.bfloat16, tag="vib")
        nc.vector.tensor_copy(vrb[:], vre[:])
        nc.vector.tensor_copy(vib[:], vim[:])
        nc.sync.dma_start(vpr_d, vrb[:])
        nc.sync.dma_start(vpi_d, vib[:])

    nc.compile()
    return nc


# ---------------------------------------------------------------- kernel 2
def build_k2():
    nc = bacc.Bacc("TRN2", target_bir_lowering=False, debug=False)
    rsp_d = nc.dram_tensor("tloc", [126, NSH], DT.bfloat16,
                           kind="ExternalInput").ap()
    ktab_d = nc.dram_tensor("sinv", [126, KPAD], DT.bfloat16,
                            kind="ExternalInput").ap()
    qT_d = nc.dram_tensor("qT", [D, NSH], DT.float32r, kind="ExternalInput").ap()
    akp_d = nc.dram_tensor("akp", [D, AWK], DT.float32r, kind="ExternalInput").ap()
    vprT_d = nc.dram_tensor("vprT", [KPAD, D], DT.bfloat16,
                            kind="ExternalInput").ap()
    vpiT_d = nc.dram_tensor("vpiT", [KPAD, D], DT.bfloat16,
                            kind="ExternalInput").ap()
    outT0_d = nc.dram_tensor("outT0", [D, NSH], DT.float32,
                             kind="ExternalOutput").ap()
    outT1_d = nc.dram_tensor("outT1", [D, NSH], DT.float32,
                             kind="ExternalOutput").ap()
    zs_d = nc.dram_tensor("zs", [P, 2 * NC2], DT.float32, kind="ExternalOutput").ap()
    mx_d = nc.dram_tensor("mxs", [P, 2 * NC2], DT.float32, kind="ExternalOutput").ap()

    with ExitStack() as ctx:
        tc = ctx.enter_context(tile.TileContext(nc))
        cpool = ctx.enter_context(tc.tile_pool(name="const", bufs=1))
        smpool = ctx.enter_context(tc.tile_pool(name="sm", bufs=1))
        wpool = ctx.enter_context(tc.tile_pool(name="work", bufs=2))
        zpool = ctx.enter_context(tc.tile_pool(name="z", bufs=1))

        rsp = cpool.tile([126, NSH], DT.bfloat16)
        ktab = cpool.tile([126, KPAD], DT.bfloat16)
        qT = cpool.tile([D, NSH], DT.float32r)
        akp = cpool.tile([D, AWK], DT.float32r)
        vprT = cpool.tile([P, KCH * D], DT.bfloat16)  # [128 k-part, chunk-major d]
        vpiT = cpool.tile([P, KCH * D], DT.bfloat16)
        ident = cpool.tile([P, P], DT.bfloat16)
        make_identity(nc, ident[:])
        # issue input DMAs from separate engine queues (parallel DGE)
        nc.sync.dma_start(rsp[:], rsp_d)
        nc.sync.dma_start(ktab[:], ktab_d)
        nc.gpsimd.dma_start(qT[:], qT_d)
        nc.scalar.dma_start(akp[:], akp_d)
        nc.gpsimd.dma_start(vprT[:].rearrange("p (c d) -> p c d", d=D),
                            vprT_d.rearrange("(c p) d -> p c d", p=P))
        nc.scalar.dma_start(vpiT[:].rearrange("p (c d) -> p c d", d=D),
                            vpiT_d.rearrange("(c p) d -> p c d", p=P))

        sm = smpool.tile([P, NC2 * AWK], DT.bfloat16)  # [128 n-part, chunk-major k]
        zacc = zpool.tile([P, 2 * NC2], DT.float32)    # [z_half0 | z_half1]
        mxout = zpool.tile([P, 2 * NC2], DT.float32)   # [-m_half0 | -m_half1]

        # ---- pass 1: aw -> unnormalized softmax per atom chunk, computed in
        # two INDEPENDENT 1920-col halves (separate max shift per half, merged
        # on the host). Halves ping-pong 4-bank PSUM tiles and have no cross
        # dependency, so the pipeline period is one reduce+exp, not a chunk.
        with tc.tile_pool(name="awps", bufs=2, space="PSUM") as awps:
            PIECES = (0, 512, 1024, 1536, AWH)
            for c in range(NC2):
                for h in range(2):
                    aw = awps.tile([P, AWH], DT.float32, tag="aw")
                    for j in range(4):
                        nc.tensor.matmul(
                            aw[:, PIECES[j]:PIECES[j + 1]],
                            qT[:, c * P:(c + 1) * P],
                            akp[:, h * AWH + PIECES[j]:h * AWH + PIECES[j + 1]],
                            start=True, stop=True)
                    negmx = mxout[:, h * NC2 + c:h * NC2 + c + 1]
                    nc.vector.reduce_max(negmx, aw[:], axis=AX.X, negate=True)
                    nc.scalar.activation(
                        sm[:, c * AWK + h * AWH:c * AWK + (h + 1) * AWH],
                        aw[:], F.Exp, bias=negmx,
                        accum_out=zacc[:, h * NC2 + c:h * NC2 + c + 1])

        # z and -m are final once pass 1 ends; DMA them out under pass 2
        nc.sync.dma_start(zs_d, zacc[:])
        nc.sync.dma_start(mx_d, mxout[:])

        # ---- pass 2: eik_i (transposed layout) + inverse transform. Two
        # sequential out.T accumulators matching the pass-1 halves (kc<15 ->
        # outT0 with shift m0, kc>=15 -> outT1 with m1); host merges.
        with (tc.tile_pool(name="phps", bufs=2, space="PSUM") as phps,
              tc.tile_pool(name="smps", bufs=2, space="PSUM") as smps,
              tc.tile_pool(name="ops", bufs=1, space="PSUM") as ops):
            HK = KCH // 2  # 15 kc per half

            def emit_front(kc):
                """ph matmuls + sm transposes for chunk kc (PE-only work,
                emitted one chunk ahead so the PE queue never blocks the
                DVE chain's inputs behind dependent out-matmuls)."""
                ph = phps.tile([P, NSH], DT.float32, tag="ph")
                for h in range(2):
                    nc.tensor.matmul(ph[:, h * 512:(h + 1) * 512],
                                     ktab[:, kc * P:(kc + 1) * P],
                                     rsp[:, h * 512:(h + 1) * 512],
                                     start=True, stop=True)
                smT = smps.tile([P, NSH], DT.bfloat16, tag="smT")
                for c in range(NC2):
                    nc.tensor.transpose(
                        smT[:, c * P:(c + 1) * P],
                        sm[:, c * AWK + kc * P: c * AWK + (kc + 1) * P],
                        ident[:])
                return ph, smT

            prev = emit_front(0)
            for half in range(2):
                outT = ops.tile([P, NSH], DT.float32, tag="outT")
                for kk in range(HK):
                    kc = half * HK + kk
                    nxt = emit_front(kc + 1) if kc + 1 < KCH else None
                    ph, smT = prev
                    # ACT copies ph to SBUF so the magic round runs in DVE 2x
                    # dual-port mode (PSUM operands disable it)
                    phs = wpool.tile([P, NSH], DT.float32, tag="phs")
                    nc.scalar.activation(phs[:], ph[:], F.Copy)
                    tr = wpool.tile([P, NSH], DT.float32, tag="tr")
                    nc.vector.tensor_scalar(tr[:], phs[:], MAGIC, MAGIC,
                                            ALU.add, ALU.subtract)
                    sc = wpool.tile([P, 2 * NSH], DT.float32, tag="sc")
                    nc.vector.scalar_tensor_tensor(sc[:, :NSH], tr[:], 0.0,
                                                   phs[:], ALU.add,
                                                   ALU.subtract)
                    nc.vector.add_range_wrap(sc[:, NSH:], sc[:, :NSH],
                                             -0.25, 0.5, 1.0)
                    sincos = wpool.tile([P, 2 * NSH], DT.bfloat16,
                                        tag="sincos")
                    nc.scalar.activation(sincos[:], sc[:], F.Sin, scale=-TWOPI)
                    smC = wpool.tile([P, NSH], DT.bfloat16, tag="smC")
                    smS = wpool.tile([P, NSH], DT.bfloat16, tag="smS")
                    nc.vector.tensor_mul(smC[:], smT[:], sincos[:, NSH:])
                    nc.vector.tensor_mul(smS[:], smT[:], sincos[:, :NSH])
                    # out.T += vprT_c.T @ smC + vpiT_c.T @ smS
                    for h in range(2):
                        hs = slice(h * 512, (h + 1) * 512)
                        nc.tensor.matmul(outT[:, hs],
                                         vprT[:, kc * D:(kc + 1) * D],
                                         smC[:, hs], start=(kk == 0),
                                         stop=False)
                        nc.tensor.matmul(outT[:, hs],
                                         vpiT[:, kc * D:(kc + 1) * D],
                                         smS[:, hs], start=False,
                                         stop=(kk == HK - 1))
                    prev = nxt
                res = wpool.tile([P, NSH], DT.float32, tag="res")
                nc.vector.tensor_copy(res[:], outT[:])
                nc.sync.dma_start(outT0_d if half == 0 else outT1_d, res[:])

    nc.compile()
    return nc


# ---------------------------------------------------------------- profiling
def enable_ntff_profiling():
    """Provide the antenv.axon_hooks module run_bass_kernel_spmd needs for
    trace=True under axon, backed by trn_boot's ctypes NTFF hook."""
    import types
    if "antenv.axon_hooks" in sys.modules:
        return True
    sys.path.insert(0, "/root/.axon_site")
    try:
        from trn_agent_boot.trn_boot import _ntff_profile_via_ctypes
        hook = _ntff_profile_via_ctypes("/opt/axon/libaxon_pjrt.so")
    except Exception as e:
        print(f"ntff hook unavailable: {e}")
        return False
    if hook is None:
        print("ntff hook: .so lacks axon_start_nrt_profile")
        return False
    mod = types.ModuleType("antenv.axon_hooks")
    mod._hook = hook
    mod.get_axon_ntff_profile_hook = lambda: mod._hook
    mod.set_axon_ntff_profile_hook = lambda h: setattr(mod, "_hook", h)
    sys.modules["antenv.axon_hooks"] = mod
    # upload_artifacts copies the NEFF dir to a remote bucket -- hangs in
    # this container; keep artifacts local instead.
    import concourse.bass_utils as bu
    bu.upload_artifacts = lambda tmpdir: tmpdir
    return True


# ---------------------------------------------------------------- runner
_NC1 = None
_NC2 = None


def run_ewald(q_vector, k_vector, v_vector, positions, cell, batch, k_fwd,
              k_inv, trace=False):
    global _NC1, _NC2
    if trace:
        trace = enable_ntff_profiling()
    tstk, Sfwd, Sinv, qT_abs, kv, vv, K = host_prep(
        q_vector, k_vector, v_vector, positions, cell, k_fwd, k_inv)

    if _NC1 is None:
        _NC1 = build_k1()
    in1 = [{"tstk": tstk,
            "sfwd": np.ascontiguousarray(Sfwd[:, c * KSH:(c + 1) * KSH]),
            "kv": kv, "vv": vv} for c in range(8)]
    r1 = run_bass_kernel_spmd(_NC1, in1, list(range(8)), trace=trace)

    akp = np.concatenate([r1.results[c]["akp"] for c in range(8)], axis=1)
    vpr = np.concatenate([r1.results[c]["vpr"] for c in range(8)], axis=1)
    vpi = np.concatenate([r1.results[c]["vpi"] for c in range(8)], axis=1)
    akp[:, K:] = 0.0
    akp_pad = np.ascontiguousarray(akp)  # AWK == KPAD
    vprT = np.ascontiguousarray(vpr.T)  # [KPAD, 128] bf16
    vpiT = np.ascontiguousarray(vpi.T)
    vprT[K:, :] = 0
    vpiT[K:, :] = 0

    if _NC2 is None:
        _NC2 = build_k2()
    in2 = [{"tloc": np.ascontiguousarray(tstk[:, c * NSH:(c + 1) * NSH]),
            "sinv": Sinv,
            "qT": np.ascontiguousarray(qT_abs[:, c * NSH:(c + 1) * NSH]),
            "akp": akp_pad, "vprT": vprT, "vpiT": vpiT} for c in range(8)]
    r2 = run_bass_kernel_spmd(_NC2, in2, list(range(8)), trace=trace)

    outs = []
    for c in range(8):
        o0 = r2.results[c]["outT0"]              # [128 d, 1024 n], shift m0
        o1 = r2.results[c]["outT1"]              # [128 d, 1024 n], shift m1
        zh = r2.results[c]["zs"]                 # [128, 16] = [z_h0 | z_h1]
        mh = r2.results[c]["mxs"]                # [128, 16] = [-m_h0 | -m_h1]
        m0 = -mh[:, :NC2].T.reshape(-1).astype(np.float64)
        m1 = -mh[:, NC2:].T.reshape(-1).astype(np.float64)
        m = np.maximum(m0, m1)
        s0 = np.exp(m0 - m)
        s1 = np.exp(m1 - m)
        z = (zh[:, :NC2].T.reshape(-1) * s0 + zh[:, NC2:].T.reshape(-1) * s1)
        num = o0.T * s0[:, None] + o1.T * s1[:, None]
        outs.append((num / z[:, None]).astype(np.float32))
    out = np.concatenate(outs, axis=0)
    return out, (r1, r2)


# ---------------------------------------------------------------- entry point
def kernel(q_vector, k_vector, v_vector, positions, cell, batch, k_fwd, k_inv):
    """Full-input entry: shards across 8 NeuronCores internally."""
    out, _ = run_ewald(np.asarray(q_vector), np.asarray(k_vector),
                       np.asarray(v_vector), np.asarray(positions),
                       np.asarray(cell), np.asarray(batch),
                       np.asarray(k_fwd), np.asarray(k_inv))
    return out
